# revision 1
# baseline (speedup 1.0000x reference)
"""GQA attention (B=4,S=1024,D=2048,H=32,KVH=8,HD=64) + RoPE, tensor-parallel
over the 8 kv-head groups across 8 NeuronCores.

Per-core pipeline (all-transposed layouts, no on-device softmax-max pass):
  qT/kT/vT = W.T @ xT            (PE, bf16, D-contraction in 16 chunks of 128)
  RoPE: qrot = (q*cos) + A@(q*sin)    (rotate-half folded into a PE matmul A)
  sT = k_ropeT.T-block @ q_ropeT      ([keys,q] scores, K=64 contraction,
                                       head pairs packed into PE row groups)
  pT = exp(sT/8)                      (ACT, one N=2048 call per key block)
  avT_aug = [v|ones64].T @ pT         (PE; rows 64:128 = 64 copies of the
                                       softmax denominator)
  avn = avT * reciprocal(denom)       (DVE only; odd head shifted to
                                       partitions 64:128 by SBUF-SBUF DMA)
  y_partial = avn_pairT.T-block @ Wo_pair  (PE, full K=128 contraction)
  DMA out bf16; host sums the 8 partials in fp32.

Program order interleaves proj(b+2) with attention(b) so the PE always has
dense matmul work queued (keeps the HAM clock gate at 8/8).
"""

import numpy as np
import ml_dtypes

import concourse.bass as bass
import concourse.mybir as mybir
import concourse.tile as tile
from concourse import bacc
from concourse import bass_utils

BF16 = mybir.dt.bfloat16
F32 = mybir.dt.float32
BF = ml_dtypes.bfloat16

B, S, D = 4, 1024, 2048
H, KVH, HD = 32, 8, 64
NREP = H // KVH          # 4 q heads per core
T = B * S                # 4096 tokens
NC = 8                   # cores
QD = NREP * HD           # 256 q dims per core
KC = D // 128            # 16 contraction chunks
TB = 512                 # proj token-block
AF = mybir.ActivationFunctionType

_CACHE = {}


def _build():
    if "nc" in _CACHE:
        return _CACHE["nc"]
    nc = bacc.Bacc("TRN2", target_bir_lowering=False)

    xT_d = nc.dram_tensor("xT", (D, T), BF16, kind="ExternalInput")
    wq_d = nc.dram_tensor("wq", (D, QD), BF16, kind="ExternalInput")
    wkv_d = nc.dram_tensor("wkv", (D, 128), BF16, kind="ExternalInput")
    wo_d = nc.dram_tensor("wo", (QD, D), BF16, kind="ExternalInput")
    cos_d = nc.dram_tensor("cos2", (128, S), F32, kind="ExternalInput")
    sin_d = nc.dram_tensor("sin2", (128, S), F32, kind="ExternalInput")
    arot_d = nc.dram_tensor("arot", (128, 128), BF16, kind="ExternalInput")
    eye_d = nc.dram_tensor("eye2", (128, 64), BF16, kind="ExternalInput")
    y_d = nc.dram_tensor("y", (T, D), BF16, kind="ExternalOutput")

    with tile.TileContext(nc) as tc:
        with (
            tc.tile_pool(name="const", bufs=1) as cpool,
            tc.tile_pool(name="qk", bufs=4) as qkpool,
            tc.tile_pool(name="kv", bufs=2) as kvpool,
            tc.tile_pool(name="vt", bufs=2) as vtpool,
            tc.tile_pool(name="va", bufs=2) as vapool,
            tc.tile_pool(name="xin", bufs=2) as xpool,
            tc.tile_pool(name="rt", bufs=3) as rpool,
            tc.tile_pool(name="pb", bufs=3) as prpool,
            tc.tile_pool(name="nm", bufs=3) as npool,
            tc.tile_pool(name="an", bufs=3) as apool,
            tc.tile_pool(name="yo", bufs=4) as ypool,
            # 8 PSUM banks total: pacc 1 (proj accum), pshv 2 (rope shift +
            # v transpose + outproj y), pscr 4 (two [128,1024] score tiles),
            # pav 1 (AV accum)
            tc.tile_pool(name="pacc", bufs=1, space="PSUM") as pacc,
            tc.tile_pool(name="pshv", bufs=2, space="PSUM") as pshv,
            tc.tile_pool(name="pscr", bufs=2, space="PSUM") as pscr,
            tc.tile_pool(name="pav", bufs=1, space="PSUM") as pav,
        ):
            # ---- constants; first wq chunks on the sync queue (ahead of x),
            # bulk on the gpsimd queue so x loads aren't stuck behind them ----
            wq_sb = cpool.tile([128, KC * QD], BF16, tag="wq")
            wq_dv = wq_d[:].rearrange("(c p) m -> p c m", p=128)
            wq_sv = wq_sb[:].rearrange("p (c m) -> p c m", c=KC)
            nc.sync.dma_start(out=wq_sv[:, 0:4, :], in_=wq_dv[:, 0:4, :])
            wkv_sb = cpool.tile([128, KC * 128], BF16, tag="wkv")
            nc.gpsimd.dma_start(
                out=wkv_sb[:].rearrange("p (c m) -> p c m", c=KC),
                in_=wkv_d[:].rearrange("(c p) m -> p c m", p=128),
            )
            cos_sb = cpool.tile([128, S], F32, tag="cos")
            nc.gpsimd.dma_start(out=cos_sb[:], in_=cos_d[:])
            sin_sb = cpool.tile([128, S], F32, tag="sin")
            nc.gpsimd.dma_start(out=sin_sb[:], in_=sin_d[:])
            arot_sb = cpool.tile([128, 128], BF16, tag="arot")
            nc.gpsimd.dma_start(out=arot_sb[:], in_=arot_d[:])
            nc.gpsimd.dma_start(out=wq_sv[:, 4:KC, :], in_=wq_dv[:, 4:KC, :])
            eye_sb = cpool.tile([128, 64], BF16, tag="eye")
            nc.gpsimd.dma_start(out=eye_sb[:], in_=eye_d[:])
            # Wo pair p (local heads 2p,2p+1) = rows p*128:(p+1)*128 -> [128, D]
            wo_sb = cpool.tile([128, 2 * D], BF16, tag="wo")
            nc.gpsimd.dma_start(
                out=wo_sb[:].rearrange("p (h n) -> p h n", h=2),
                in_=wo_d[:].rearrange("(h p) n -> p h n", p=128),
            )

            qrope = {}   # (b, pr) -> [128, S] bf16 (head 2pr rows 0:64, 2pr+1 rows 64:128)
            kT = {}      # b -> [128, S] bf16 (k_rope duplicated top/bottom)
            v_aug = {}   # b -> [128, 8*128] bf16 ([v | ones64] per key block)
            avn = {}     # (b, pr) -> [128, S] bf16 normalized AV pair

            def emit_proj(b):
                vT_t = vtpool.tile([128, S], BF16, tag="vT")
                kT_t = kvpool.tile([128, S], BF16, tag="kT")
                q_t = [qkpool.tile([128, S], BF16, tag="qr", name=f"qr{b}_{p}")
                       for p in range(2)]
                qrope[b] = q_t
                kT[b] = kT_t
                for half in range(2):
                    tb = 2 * b + half
                    scol = half * TB
                    xts = xpool.tile([128, KC * TB], BF16, tag="xts")
                    xv = xts[:].rearrange("p (c n) -> p c n", c=KC)
                    xdv = xT_d[:, bass.ts(tb, TB)].rearrange("(c p) n -> p c n", p=128)
                    if tb == 0:
                        # fine-grained first load, alternating DMA queues so
                        # the pieces transfer in parallel rings
                        for c4 in range(0, KC, 4):
                            nc.sync.dma_start(
                                out=xv[:, c4:c4 + 4, :], in_=xdv[:, c4:c4 + 4, :]
                            )
                    else:
                        nc.sync.dma_start(out=xv[:, 0:8, :], in_=xdv[:, 0:8, :])
                        nc.sync.dma_start(out=xv[:, 8:KC, :], in_=xdv[:, 8:KC, :])
                    css = cos_sb[:, scol:scol + TB]
                    sns = sin_sb[:, scol:scol + TB]
                    for st in range(3):
                        acc = pacc.tile([128, TB], F32, tag="acc", name=f"acc{tb}_{st}")
                        for c in range(KC):
                            if st < 2:
                                w = wq_sb[:, c * QD + st * 128:c * QD + (st + 1) * 128]
                            else:
                                w = wkv_sb[:, bass.ts(c, 128)]
                            nc.tensor.matmul(
                                acc[:], w, xts[:, bass.ts(c, TB)],
                                start=(c == 0), stop=(c == KC - 1),
                            )
                        if st < 2:
                            qsin = rpool.tile([128, TB], BF16, tag="qsin")
                            nc.vector.tensor_mul(qsin[:], acc[:], sns)
                            t1 = rpool.tile([128, TB], BF16, tag="t1")
                            nc.vector.tensor_mul(t1[:], acc[:], css)
                            sh = pshv.tile([128, TB], F32, tag="shv", name=f"sh{tb}_{st}")
                            nc.tensor.matmul(sh[:], arot_sb[:], qsin[:], start=True, stop=True)
                            nc.vector.tensor_add(q_t[st][:, scol:scol + TB], t1[:], sh[:])
                        else:
                            ksin = rpool.tile([64, TB], BF16, tag="qsin")
                            nc.vector.tensor_mul(ksin[:], acc[0:64, :], sns[0:64])
                            t1k = rpool.tile([64, TB], BF16, tag="t1")
                            nc.vector.tensor_mul(t1k[:], acc[0:64, :], css[0:64])
                            sh = pshv.tile([128, TB], F32, tag="shv", name=f"sh{tb}_{st}")
                            nc.tensor.matmul(
                                sh[0:64, :], arot_sb[0:64, 0:64], ksin[:],
                                start=True, stop=True,
                            )
                            nc.vector.tensor_add(kT_t[0:64, scol:scol + TB], t1k[:], sh[0:64, :])
                            nc.sync.dma_start(
                                out=kT_t[64:128, scol:scol + TB],
                                in_=kT_t[0:64, scol:scol + TB],
                            )
                            nc.vector.tensor_copy(vT_t[64:128, scol:scol + TB], acc[64:128, :])
                # v natural (+ ones block) per key block of 128
                va_t = vapool.tile([128, 8 * 128], BF16, tag="vaug")
                v_aug[b] = va_t
                vav = va_t[:].rearrange("p (k c) -> p k c", k=8)
                nc.vector.memset(vav[:, :, 64:128], 1.0)
                for kb in range(8):
                    vtr = pshv.tile([128, 64], BF16, tag="shv", name=f"vtr{b}_{kb}")
                    nc.tensor.transpose(
                        vtr[:], vT_t[64:128, kb * 128:(kb + 1) * 128], eye_sb[64:128, :]
                    )
                    nc.vector.tensor_copy(vav[:, kb, 0:64], vtr[:])

            def emit_attn(b, pr):
                probs = prpool.tile([128, 8 * 2048], BF16, tag="probs")
                q_t = qrope[b][pr]
                for kb in range(8):
                    # one scores tile per head half so exp(kb) overlaps
                    # scores(kb+1); matmuls interleaved across PE row groups
                    sA = pscr.tile([128, 1024], F32, tag="scr", name=f"sA{b}{pr}{kb}")
                    sB = pscr.tile([128, 1024], F32, tag="scr", name=f"sB{b}{pr}{kb}")
                    for qh in range(2):
                        for hh, s_t in ((0, sA), (1, sB)):
                            r0 = hh * 64
                            nc.tensor.matmul(
                                s_t[:, qh * 512:(qh + 1) * 512],
                                kT[b][r0:r0 + 64, kb * 128:(kb + 1) * 128],
                                q_t[r0:r0 + 64, qh * 512:(qh + 1) * 512],
                                start=True, stop=True,
                            )
                    nc.scalar.activation(
                        probs[:, kb * 2048:kb * 2048 + 1024], sA[:], AF.Exp, scale=0.125,
                    )
                    nc.scalar.activation(
                        probs[:, kb * 2048 + 1024:(kb + 1) * 2048], sB[:], AF.Exp, scale=0.125,
                    )
                avn_t = apool.tile([128, S], BF16, tag="avn")
                for h in range(2):
                    for qh in range(2):
                        if b == 3 and (h * 2 + qh) % 2 == 1:
                            # proj accum bank is idle by batch 3: borrow it so
                            # consecutive AV streams double-buffer in the tail
                            avp = pacc.tile([128, 512], F32, tag="acc",
                                            name=f"avp{b}{pr}{h}{qh}")
                        else:
                            avp = pav.tile([128, 512], F32, tag="av",
                                           name=f"avp{b}{pr}{h}{qh}")
                        for kb in range(8):
                            nc.tensor.matmul(
                                avp[:],
                                v_aug[b][:, kb * 128:(kb + 1) * 128],
                                probs[:, kb * 2048 + h * 1024 + qh * 512:
                                      kb * 2048 + h * 1024 + (qh + 1) * 512],
                                start=(kb == 0), stop=(kb == 7),
                            )
                        den = npool.tile([64, 512], F32, tag="den")
                        nc.vector.tensor_copy(den[:], avp[64:128, :])
                        rbc = npool.tile([64, 512], F32, tag="rbc")
                        nc.vector.reciprocal_approx_fast(rbc[:], den[:])
                        if h == 0:
                            nc.vector.tensor_mul(
                                avn_t[0:64, qh * 512:(qh + 1) * 512], avp[0:64, :], rbc[:]
                            )
                        else:
                            aodd = npool.tile([64, 512], BF16, tag="aodd")
                            nc.vector.tensor_mul(aodd[:], avp[0:64, :], rbc[:])
                            nc.gpsimd.dma_start(
                                out=avn_t[64:128, qh * 512:(qh + 1) * 512], in_=aodd[:]
                            )
                avn[(b, pr)] = avn_t

            def emit_outproj(b):
                for t in range(8):
                    for nb in range(4):
                        if b == 3 and (t * 4 + nb) % 2 == 1:
                            # scores banks are idle in the tail: borrow them so
                            # the outproj evacuation rotates 4-deep
                            yp = pscr.tile([128, 512], F32, tag="scr",
                                           name=f"yp{b}{t}{nb}")
                        else:
                            yp = pshv.tile([128, 512], F32, tag="shv",
                                           name=f"yp{b}{t}{nb}")
                        for p in range(2):
                            nc.tensor.matmul(
                                yp[:],
                                avn[(b, p)][:, t * 128:(t + 1) * 128],
                                wo_sb[:, p * D + nb * 512:p * D + (nb + 1) * 512],
                                start=(p == 0), stop=(p == 1),
                            )
                        ys = ypool.tile([128, 512], BF16, tag="ys")
                        # split PSUM->SBUF evacuation across ACT and DVE
                        if nb % 2 == 0:
                            nc.scalar.copy(ys[:], yp[:])
                        else:
                            nc.vector.tensor_copy(ys[:], yp[:])
                        nc.gpsimd.dma_start(
                            out=y_d[b * S + t * 128:b * S + (t + 1) * 128,
                                    nb * 512:(nb + 1) * 512],
                            in_=ys[:],
                        )

            # Interleave proj(b+2) and outproj(b-1) with attention(b) so the
            # PE always has ready matmul work; pull attn(3) ahead of
            # outproj(2) so the tail keeps multiple engines fed.
            emit_proj(0)
            emit_proj(1)
            emit_attn(0, 0)
            emit_attn(0, 1)
            emit_proj(2)
            emit_outproj(0)
            emit_attn(1, 0)
            emit_attn(1, 1)
            emit_proj(3)
            emit_outproj(1)
            emit_attn(2, 0)
            emit_attn(2, 1)
            emit_attn(3, 0)
            emit_outproj(2)
            emit_attn(3, 1)
            emit_outproj(3)

    nc.compile()
    _CACHE["nc"] = nc
    return nc


def _host_prep(x, cos, sin, Wq, Wk, Wv, Wo):
    x = np.asarray(x, np.float32)
    xT = np.ascontiguousarray(x.reshape(T, D).T).astype(BF)
    cosT = np.asarray(cos, np.float32).T
    sinT = np.asarray(sin, np.float32).T
    cos2 = np.ascontiguousarray(np.tile(cosT, (2, 1)))          # (128, S) f32
    sin2 = np.ascontiguousarray(np.tile(sinT, (2, 1)))
    # lhsT for qshiftT = A @ qT  ->  arot = A.T (block-diag x2 over heads)
    A = np.zeros((HD, HD), np.float32)
    for d in range(32):
        A[d, d + 32] = -1.0
        A[32 + d, d] = 1.0
    arot = np.kron(np.eye(2, dtype=np.float32), A.T).astype(BF)  # (128,128)
    eye2 = np.tile(np.eye(64, dtype=np.float32), (2, 1)).astype(BF)  # (128,64)

    Wq = np.asarray(Wq, np.float32)
    Wk = np.asarray(Wk, np.float32)
    Wv = np.asarray(Wv, np.float32)
    Wo = np.asarray(Wo, np.float32)
    in_maps = []
    for g in range(NC):
        wq_g = np.ascontiguousarray(Wq[:, g * QD:(g + 1) * QD]).astype(BF)
        wkv_g = np.ascontiguousarray(
            np.concatenate([Wk[:, g * HD:(g + 1) * HD], Wv[:, g * HD:(g + 1) * HD]], axis=1)
        ).astype(BF)
        wo_g = np.ascontiguousarray(Wo[g * QD:(g + 1) * QD, :]).astype(BF)
        in_maps.append({
            "xT": xT, "wq": wq_g, "wkv": wkv_g, "wo": wo_g,
            "cos2": cos2, "sin2": sin2, "arot": arot, "eye2": eye2,
        })
    return in_maps


def kernel(x, cos, sin, Wq, Wk, Wv, Wo):
    nc = _build()
    in_maps = _host_prep(x, cos, sin, Wq, Wk, Wv, Wo)
    res = bass_utils.run_bass_kernel_spmd(
        nc, in_maps, core_ids=list(range(NC)), trace=False,
    )
    y = np.zeros((T, D), np.float32)
    for r in res.results:
        y += np.asarray(r["y"], np.float32)
    return y.reshape(B, S, D)



# revision 9
# speedup vs baseline: 1.2328x; 1.2328x over previous
"""GQA attention (B=4,S=1024,D=2048,H=32,KVH=8,HD=64) + RoPE, tensor-parallel
over the 8 kv-head groups across 8 NeuronCores.

Per-core pipeline (all-transposed layouts, no on-device softmax-max pass):
  qT/kT/vT = W.T @ xT            (PE, bf16, D-contraction in 16 chunks of 128)
  RoPE: qrot = (q*cos) + A@(q*sin)    (rotate-half folded into a PE matmul A)
  attention inner loop per (qh-half, key-block):
    sAB[:, 0:512]   = k.T @ q_head0    (tile_position (0,0),  K=64 rows 0:63)
    sAB[:, 512:1024]= k.T @ q_head1    (tile_position (64,0), K=64 rows 64:127)
    pAB = exp(sAB/8)                   (one ACT call per key-block)
    avA += [v|ones].T @ pAB[:,0:512]   (PE accumulation chains, 2 banks)
    avB += [v|ones].T @ pAB[:,512:1024]
  avn = avT * reciprocal(denom rows)   (DVE; odd head shifted to partitions
                                        64:128 by gpsimd SBUF-SBUF DMA)
  y_partial = avn_pairT.T @ Wo_pair    (PE, K=128 contraction, PSUM banks
                                        borrowed from the finished proj pools)
  y staged to [128, 2048] rows, stored by ring DMA on sync/scalar queues;
  host sums the 8 partials in fp32.

Program order interleaves proj(b+2)/outproj(b-1) with attention(b) so the PE
always has dense matmul work while ACT runs the exp stream.
"""

import numpy as np
import ml_dtypes

import concourse.bass as bass
import concourse.mybir as mybir
import concourse.tile as tile
from concourse import bacc
from concourse import bass_utils

BF16 = mybir.dt.bfloat16
F32 = mybir.dt.float32
BF = ml_dtypes.bfloat16

B, S, D = 4, 1024, 2048
H, KVH, HD = 32, 8, 64
NREP = H // KVH          # 4 q heads per core
T = B * S                # 4096 tokens
NC = 8                   # cores
QD = NREP * HD           # 256 q dims per core
KC = D // 128            # 16 contraction chunks
TB = 512                 # proj token-block
AF = mybir.ActivationFunctionType

_CACHE = {}


def _build():
    if "nc" in _CACHE:
        return _CACHE["nc"]
    nc = bacc.Bacc("TRN2", target_bir_lowering=False)

    xT_d = nc.dram_tensor("xT", (D, T), BF16, kind="ExternalInput")
    wq_d = nc.dram_tensor("wq", (D, QD), BF16, kind="ExternalInput")
    wkv_d = nc.dram_tensor("wkv", (D, 128), BF16, kind="ExternalInput")
    wo_d = nc.dram_tensor("wo", (QD, D), BF16, kind="ExternalInput")
    cos_d = nc.dram_tensor("cos2", (128, S), F32, kind="ExternalInput")
    sin_d = nc.dram_tensor("sin2", (128, S), F32, kind="ExternalInput")
    arot_d = nc.dram_tensor("arot", (128, 128), BF16, kind="ExternalInput")
    eye_d = nc.dram_tensor("eye2", (128, 64), BF16, kind="ExternalInput")
    y_d = nc.dram_tensor("y", (T, D), BF16, kind="ExternalOutput")

    with tile.TileContext(nc) as tc:
        with (
            tc.tile_pool(name="const", bufs=1) as cpool,
            tc.tile_pool(name="qk", bufs=4) as qkpool,
            tc.tile_pool(name="kv", bufs=2) as kvpool,
            tc.tile_pool(name="vt", bufs=2) as vtpool,
            tc.tile_pool(name="va", bufs=2) as vapool,
            tc.tile_pool(name="xin", bufs=4) as xpool,
            tc.tile_pool(name="rt", bufs=3) as rpool,
            tc.tile_pool(name="pb", bufs=4) as prpool,
            tc.tile_pool(name="nm", bufs=4) as npool,
            tc.tile_pool(name="an", bufs=4) as apool,
            tc.tile_pool(name="yo", bufs=3) as ypool,
            # 8 PSUM banks total: pacc 1 (proj accum / outproj), pshv 1 (rope
            # shift + v transpose / outproj), pscr 4 (two [128,1024] score
            # tiles, depth-2 rotation over kb), pav 2 (AV accumulation chains)
            tc.tile_pool(name="pacc", bufs=1, space="PSUM") as pacc,
            tc.tile_pool(name="pshv", bufs=1, space="PSUM") as pshv,
            tc.tile_pool(name="pscr", bufs=2, space="PSUM") as pscr,
            tc.tile_pool(name="pav", bufs=2, space="PSUM") as pav,
        ):
            # ---- constants + x, ordered by first use. sync and scalar are
            # both HW DGE queues (ring DMAs); gpsimd takes the late consts ----
            wq_sb = cpool.tile([128, KC * QD], BF16, tag="wq")
            wq_dv = wq_d[:].rearrange("(c p) m -> p c m", p=128)
            wq_sv = wq_sb[:].rearrange("p (c m) -> p c m", c=KC)
            wkv_sb = cpool.tile([128, KC * 128], BF16, tag="wkv")
            cos_sb = cpool.tile([128, S], F32, tag="cos")
            sin_sb = cpool.tile([128, S], F32, tag="sin")

            nc.sync.dma_start(out=wq_sv[:, 0:4, :], in_=wq_dv[:, 0:4, :])
            nc.scalar.dma_start(
                out=wkv_sb[:].rearrange("p (c m) -> p c m", c=KC),
                in_=wkv_d[:].rearrange("(c p) m -> p c m", p=128),
            )

            xts_all = {}

            def load_x(tb, queue, fine=False):
                xts = xpool.tile([128, KC * TB], BF16, tag="xts", name=f"x{tb}")
                xts_all[tb] = xts
                xv = xts[:].rearrange("p (c n) -> p c n", c=KC)
                xdv = xT_d[:, bass.ts(tb, TB)].rearrange("(c p) n -> p c n", p=128)
                if fine:
                    for c4 in range(0, KC, 4):
                        queue.dma_start(out=xv[:, c4:c4 + 4, :], in_=xdv[:, c4:c4 + 4, :])
                else:
                    queue.dma_start(out=xv[:, 0:8, :], in_=xdv[:, 0:8, :])
                    queue.dma_start(out=xv[:, 8:KC, :], in_=xdv[:, 8:KC, :])

            load_x(0, nc.sync, fine=True)
            nc.sync.dma_start(out=wq_sv[:, 4:10, :], in_=wq_dv[:, 4:10, :])
            nc.scalar.dma_start(out=wq_sv[:, 10:KC, :], in_=wq_dv[:, 10:KC, :])
            nc.scalar.dma_start(out=cos_sb[:, 0:TB], in_=cos_d[:, 0:TB])
            nc.scalar.dma_start(out=sin_sb[:, 0:TB], in_=sin_d[:, 0:TB])
            load_x(1, nc.sync)
            load_x(2, nc.scalar)
            nc.scalar.dma_start(out=cos_sb[:, TB:S], in_=cos_d[:, TB:S])
            nc.scalar.dma_start(out=sin_sb[:, TB:S], in_=sin_d[:, TB:S])
            load_x(3, nc.sync)

            arot_sb = cpool.tile([128, 128], BF16, tag="arot")
            nc.gpsimd.dma_start(out=arot_sb[:], in_=arot_d[:])
            eye_sb = cpool.tile([128, 64], BF16, tag="eye")
            nc.gpsimd.dma_start(out=eye_sb[:], in_=eye_d[:])
            # Wo pair p (local heads 2p,2p+1) = rows p*128:(p+1)*128 -> [128, D]
            # (tile allocated here; its DMA is emitted after proj(1) so the
            # 2MB load doesn't block kT-dup copies on the gpsimd queue)
            wo_sb = cpool.tile([128, 2 * D], BF16, tag="wo")

            def load_wo():
                nc.gpsimd.dma_start(
                    out=wo_sb[:].rearrange("p (h n) -> p h n", h=2),
                    in_=wo_d[:].rearrange("(h p) n -> p h n", p=128),
                )

            qrope = {}   # (b, pr) -> [128, S] bf16 (head 2pr rows 0:64, 2pr+1 rows 64:128)
            kT = {}      # b -> [128, S] bf16 (k_rope duplicated top/bottom)
            v_aug = {}   # b -> [128, 8*128] bf16 ([v | ones64] per key block)
            avn = {}     # (b, pr) -> [128, S] bf16 normalized AV pair

            def emit_proj(b):
                vT_t = vtpool.tile([128, S], BF16, tag="vT")
                kT_t = kvpool.tile([128, S], BF16, tag="kT")
                q_t = [qkpool.tile([128, S], BF16, tag="qr", name=f"qr{b}_{p}")
                       for p in range(2)]
                qrope[b] = q_t
                kT[b] = kT_t
                for half in range(2):
                    tb = 2 * b + half
                    scol = half * TB
                    xts = xts_all[tb]
                    css = cos_sb[:, scol:scol + TB]
                    sns = sin_sb[:, scol:scol + TB]
                    for st in range(3):
                        acc = pacc.tile([128, TB], F32, tag="acc", name=f"acc{tb}_{st}")
                        for c in range(KC):
                            if st < 2:
                                w = wq_sb[:, c * QD + st * 128:c * QD + (st + 1) * 128]
                            else:
                                w = wkv_sb[:, bass.ts(c, 128)]
                            nc.tensor.matmul(
                                acc[:], w, xts[:, bass.ts(c, TB)],
                                start=(c == 0), stop=(c == KC - 1),
                            )
                        if st < 2:
                            qsin = rpool.tile([128, TB], BF16, tag="qsin")
                            nc.vector.tensor_mul(qsin[:], acc[:], sns)
                            t1 = rpool.tile([128, TB], BF16, tag="t1")
                            nc.vector.tensor_mul(t1[:], acc[:], css)
                            sh = pshv.tile([128, TB], F32, tag="shv", name=f"sh{tb}_{st}")
                            nc.tensor.matmul(sh[:], arot_sb[:], qsin[:], start=True, stop=True)
                            nc.vector.tensor_add(q_t[st][:, scol:scol + TB], t1[:], sh[:])
                        else:
                            ksin = rpool.tile([64, TB], BF16, tag="qsin")
                            nc.vector.tensor_mul(ksin[:], acc[0:64, :], sns[0:64])
                            t1k = rpool.tile([64, TB], BF16, tag="t1")
                            nc.vector.tensor_mul(t1k[:], acc[0:64, :], css[0:64])
                            sh = pshv.tile([128, TB], F32, tag="shv", name=f"sh{tb}_{st}")
                            nc.tensor.matmul(
                                sh[0:64, :], arot_sb[0:64, 0:64], ksin[:],
                                start=True, stop=True,
                            )
                            nc.vector.tensor_add(kT_t[0:64, scol:scol + TB], t1k[:], sh[0:64, :])
                            nc.gpsimd.dma_start(
                                out=kT_t[64:128, scol:scol + TB],
                                in_=kT_t[0:64, scol:scol + TB],
                            )
                            nc.vector.tensor_copy(vT_t[64:128, scol:scol + TB], acc[64:128, :])
                    # prefetch x for the next-but-one proj while this one runs
                    nx = tb + 4
                    if 4 <= nx < 2 * B:
                        load_x(nx, nc.sync)
                # v natural (+ ones block) per key block of 128
                va_t = vapool.tile([128, 8 * 128], BF16, tag="vaug")
                v_aug[b] = va_t
                vav = va_t[:].rearrange("p (k c) -> p k c", k=8)
                nc.vector.memset(vav[:, :, 64:128], 1.0)
                for kb in range(8):
                    vtr = pshv.tile([128, 64], BF16, tag="shv", name=f"vtr{b}_{kb}")
                    nc.tensor.transpose(
                        vtr[:], vT_t[64:128, kb * 128:(kb + 1) * 128], eye_sb[64:128, :]
                    )
                    nc.vector.tensor_copy(vav[:, kb, 0:64], vtr[:])

            def emit_attn_qh(b, pr, qh):
                # one qh-half (512 q tokens): per key-block, paired score
                # matmuls on distinct PE row groups -> one exp -> two AV
                # accumulation matmuls. pscr rotates depth-2 over kb.
                q_t = qrope[b][pr]
                col = qh * 512
                if qh == 0:
                    avn[(b, pr)] = apool.tile([128, S], BF16, tag="avn",
                                              name=f"avn{b}_{pr}")
                avn_t = avn[(b, pr)]
                avA = pav.tile([128, 512], F32, tag="av", name=f"avA{b}{pr}{qh}")
                avB = pav.tile([128, 512], F32, tag="av", name=f"avB{b}{pr}{qh}")
                for kb in range(8):
                    sAB = pscr.tile([128, 1024], F32, tag="scr", name=f"s{b}{pr}{qh}{kb}")
                    nc.tensor.matmul(
                        sAB[:, 0:512],
                        kT[b][0:64, kb * 128:(kb + 1) * 128],
                        q_t[0:64, col:col + 512],
                        start=True, stop=True, tile_position=(0, 0),
                    )
                    nc.tensor.matmul(
                        sAB[:, 512:1024],
                        kT[b][64:128, kb * 128:(kb + 1) * 128],
                        q_t[64:128, col:col + 512],
                        start=True, stop=True, tile_position=(64, 0),
                    )
                    pAB = prpool.tile([128, 1024], BF16, tag="probs",
                                      name=f"p{b}{pr}{qh}{kb}")
                    nc.scalar.activation(pAB[:], sAB[:], AF.Exp, scale=0.125)
                    nc.tensor.matmul(
                        avA[:], v_aug[b][:, kb * 128:(kb + 1) * 128],
                        pAB[:, 0:512], start=(kb == 0), stop=(kb == 7),
                    )
                    nc.tensor.matmul(
                        avB[:], v_aug[b][:, kb * 128:(kb + 1) * 128],
                        pAB[:, 512:1024], start=(kb == 0), stop=(kb == 7),
                    )
                for h, avp in ((0, avA), (1, avB)):
                    den = npool.tile([64, 512], F32, tag="den")
                    nc.vector.tensor_copy(den[:], avp[64:128, :])
                    rbc = npool.tile([64, 512], F32, tag="rbc")
                    nc.vector.reciprocal_approx_fast(rbc[:], den[:])
                    if h == 0:
                        nc.vector.tensor_mul(
                            avn_t[0:64, col:col + 512], avp[0:64, :], rbc[:]
                        )
                    else:
                        aodd = npool.tile([64, 512], BF16, tag="aodd")
                        nc.vector.tensor_mul(aodd[:], avp[0:64, :], rbc[:])
                        nc.gpsimd.dma_start(
                            out=avn_t[64:128, col:col + 512], in_=aodd[:]
                        )

            def emit_attn(b, pr):
                emit_attn_qh(b, pr, 0)
                emit_attn_qh(b, pr, 1)

            def emit_outproj(b, trange):
                # yp tiles rotate over the pacc/pshv banks (proj is done or
                # far ahead); y rows staged to SBUF then stored with one big
                # ring DMA per [128, 2048] block, alternating HW DGE queues.
                for t in trange:
                    ys = ypool.tile([128, D], BF16, tag="ys", name=f"ys{b}_{t}")
                    for nb in range(4):
                        pool = pacc if nb % 2 == 0 else pshv
                        yp = pool.tile([128, 512], F32, tag="acc" if nb % 2 == 0 else "shv",
                                       name=f"yp{b}{t}{nb}")
                        for p in range(2):
                            nc.tensor.matmul(
                                yp[:],
                                avn[(b, p)][:, t * 128:(t + 1) * 128],
                                wo_sb[:, p * D + nb * 512:p * D + (nb + 1) * 512],
                                start=(p == 0), stop=(p == 1),
                            )
                        # split PSUM->SBUF evacuation across ACT and DVE
                        if nb % 2 == 0:
                            nc.scalar.copy(ys[:, nb * 512:(nb + 1) * 512], yp[:])
                        else:
                            nc.vector.tensor_copy(ys[:, nb * 512:(nb + 1) * 512], yp[:])
                    nc.sync.dma_start(
                        out=y_d[b * S + t * 128:b * S + (t + 1) * 128, :],
                        in_=ys[:],
                    )

            # Interleave proj(b+2) and outproj(b-1) with attention(b) so the
            # PE always has ready matmul work while ACT runs exp; the b=3
            # outproj is split so its first half overlaps attn(3,1).
            emit_proj(0)
            emit_proj(1)
            load_wo()
            emit_attn(0, 0)
            emit_attn(0, 1)
            emit_proj(2)
            emit_attn(1, 0)
            emit_proj(3)
            emit_attn(1, 1)
            emit_outproj(0, range(8))
            emit_attn(2, 0)
            emit_outproj(1, range(8))
            emit_attn(2, 1)
            emit_outproj(2, range(8))
            emit_attn(3, 0)
            emit_attn_qh(3, 1, 0)
            emit_outproj(3, range(4))
            emit_attn_qh(3, 1, 1)
            emit_outproj(3, range(4, 8))

    nc.compile()
    _CACHE["nc"] = nc
    return nc


def _host_prep(x, cos, sin, Wq, Wk, Wv, Wo):
    x = np.asarray(x, np.float32)
    xT = np.ascontiguousarray(x.reshape(T, D).T).astype(BF)
    cosT = np.asarray(cos, np.float32).T
    sinT = np.asarray(sin, np.float32).T
    cos2 = np.ascontiguousarray(np.tile(cosT, (2, 1)))          # (128, S) f32
    sin2 = np.ascontiguousarray(np.tile(sinT, (2, 1)))
    # lhsT for qshiftT = A @ qT  ->  arot = A.T (block-diag x2 over heads)
    A = np.zeros((HD, HD), np.float32)
    for d in range(32):
        A[d, d + 32] = -1.0
        A[32 + d, d] = 1.0
    arot = np.kron(np.eye(2, dtype=np.float32), A.T).astype(BF)  # (128,128)
    eye2 = np.tile(np.eye(64, dtype=np.float32), (2, 1)).astype(BF)  # (128,64)

    Wq = np.asarray(Wq, np.float32)
    Wk = np.asarray(Wk, np.float32)
    Wv = np.asarray(Wv, np.float32)
    Wo = np.asarray(Wo, np.float32)
    in_maps = []
    for g in range(NC):
        wq_g = np.ascontiguousarray(Wq[:, g * QD:(g + 1) * QD]).astype(BF)
        wkv_g = np.ascontiguousarray(
            np.concatenate([Wk[:, g * HD:(g + 1) * HD], Wv[:, g * HD:(g + 1) * HD]], axis=1)
        ).astype(BF)
        wo_g = np.ascontiguousarray(Wo[g * QD:(g + 1) * QD, :]).astype(BF)
        in_maps.append({
            "xT": xT, "wq": wq_g, "wkv": wkv_g, "wo": wo_g,
            "cos2": cos2, "sin2": sin2, "arot": arot, "eye2": eye2,
        })
    return in_maps


def kernel(x, cos, sin, Wq, Wk, Wv, Wo):
    nc = _build()
    in_maps = _host_prep(x, cos, sin, Wq, Wk, Wv, Wo)
    res = bass_utils.run_bass_kernel_spmd(
        nc, in_maps, core_ids=list(range(NC)), trace=False,
    )
    y = np.zeros((T, D), np.float32)
    for r in res.results:
        y += np.asarray(r["y"], np.float32)
    return y.reshape(B, S, D)
